# revision 26
# baseline (speedup 1.0000x reference)
"""Trainium2 Bass kernel for DKWinners (overlapping-window k-winners masking).

Problem: x [512, 65536] f32. boosted = x * exp((1/16 - duty_cycle) * bs).
For each of 4096 windows (window k covers boosted cols [15k, 15k+16)),
find the max; the output keeps x[:, 16k+j] where j is the argmax slot of
window k (mask laid on the non-overlapping 16-grid) and zeros the rest.

Sharding (8 cores): 4 batch-quarters x 2 column-halves. Each core handles
128 batch rows (= the 128 SBUF partitions) and 2048 windows (one column
half). Column half h covers boosted cols [30720h, 30720h+30721) and out
cols [32768h, 32768h+32768); the per-core x shard is x[:, 30720h : +34816]
which contains both.

Engine schedule: a 4-stage software pipeline over window chunks with a
3-iteration stagger so no engine ever waits on another engine's output
from the same iteration:
  stage 1 (iter i):   PE broadcasts boost factors for chunk i+1 into PSUM
                      (K=3 bf16 matmul, exact fp32 sum); ACT mirrors the
                      GPS-assigned tail into SBUF (GPS can't read PSUM).
  stage 2 (iter i):   DVE computes the boost head b=x*f from PSUM, GPS the
                      tail from the ACT mirror (split tuned so both
                      engines finish together).
  stage 3 (iter i):   DVE reduces chunk i-1's windows to maxes M (strided
                      3D view) and writes chunk i-2's winner mask o via
                      is_equal(b_win, M broadcast) on the 16-grid
                      (fp32-exact, so equality == argmax).
  stage 4 (iter i):   GPS multiplies chunk i-3's mask by x in place; ACT's
                      HWDGE queue DMAs it out.
All x stripes are issued up front on the sync queue (SBUF holds the whole
shard); out stores go on ACT's queue so loads and stores stream on
different queues. The out grid sits +2048 cols relative to the boost grid
on upper-half cores; that offset is runtime-computed from partition_id so
one SPMD program serves all 8 cores.
"""
import numpy as np
from contextlib import ExitStack

BATCH = 512
N = 65536
OUT_DIM = 4096
DPC = 16
NCORES = 8

H_WINDOWS = 2048          # windows per column half
# 96-window chunks keep the f broadcast within 3 PSUM banks so the window
# maxes M fit in the remaining 2 banks (is_equal's stride-0 broadcast of M
# then reads PSUM, not SBUF — SBUF-port contention with GPS made it 1.7x
# slower otherwise).
WCMAX = 96
# chunk plan: (window_start, window_count); tapered at the start to fill
# the pipeline quickly
_sizes = [32, 64] + [96] * 20 + [32]
assert sum(_sizes) == H_WINDOWS
CHUNKS = []
_w = 0
for _s in _sizes:
    CHUNKS.append((_w, _s))
    _w += _s
NCH = len(CHUNKS)
LBMAX = 15 * WCMAX + 1
BOOST_DVE_FRAC = 0.61     # fraction of boost cols multiplied on DVE
OUT_GPS_FRAC = 1.0        # fraction of out windows multiplied on GPS
XS_COLS = 34816           # per-core x shard cols
OUT_COLS = 32768          # per-core out cols
FS_COLS = 15 * H_WINDOWS + 1  # boost-factor cols per half (30721)
# x stripe plan: small leading stripes so chunk 0 starts early
STRIPES = [544, 1632] + [2176] * 15
assert sum(STRIPES) == XS_COLS

_CACHE: dict = {}


def _build():
    import concourse.bacc as bacc
    import concourse.bass as bass
    import concourse.mybir as mybir
    import concourse.tile as tile
    from concourse.ap import AP

    f32 = mybir.dt.float32
    bf16 = mybir.dt.bfloat16

    nc = bacc.Bacc(
        "TRN2", target_bir_lowering=False, debug=False, num_devices=NCORES
    )
    xs = nc.dram_tensor("xs", [128, XS_COLS], f32, kind="ExternalInput")
    fs3 = nc.dram_tensor("fs3", [3, FS_COLS], bf16, kind="ExternalInput")
    out = nc.dram_tensor("out", [128, OUT_COLS], f32, kind="ExternalOutput")

    def win_view(ap, w0, nwin):
        """[128, nwin, 16] overlapping-window view (stride 15), starting at
        window w0 of the buffer AP."""
        base = ap[:, 15 * w0 : 15 * w0 + 15 * nwin + 1]
        return AP(base.tensor, base.offset,
                  [list(base.ap[0]), [15, nwin], [1, DPC]])

    def grid_view(ap, w0, nwin):
        """[128, nwin, 16] contiguous 16-grid view starting at window w0."""
        base = ap[:, 16 * w0 : 16 * (w0 + nwin)]
        return AP(base.tensor, base.offset,
                  [list(base.ap[0]), [DPC, nwin], [1, DPC]])

    MIRMAX = LBMAX - int(LBMAX * BOOST_DVE_FRAC) + 8

    with tile.TileContext(nc) as tc, ExitStack() as ctx:
        bpool = ctx.enter_context(tc.tile_pool(name="b", bufs=3))
        opool = ctx.enter_context(tc.tile_pool(name="o", bufs=4))
        fpool = ctx.enter_context(tc.tile_pool(name="f", bufs=3))
        fcpool = ctx.enter_context(tc.tile_pool(name="fc", bufs=2))
        psum = ctx.enter_context(tc.tile_pool(name="ps", bufs=2, space="PSUM"))

        # Static allocations outside the pools (no pool padding).
        xs_sb = nc.alloc_sbuf_tensor("xs_sb", [128, XS_COLS], f32).ap()
        ones = nc.alloc_sbuf_tensor("ones_sb", [3, 128], bf16).ap()
        nc.vector.memset(ones, 1.0)

        stripe_off = [0]
        for w in STRIPES:
            stripe_off.append(stripe_off[-1] + w)

        def load_stripe(s):
            nc.sync.dma_start(
                xs_sb[:, stripe_off[s] : stripe_off[s + 1]],
                xs[:, stripe_off[s] : stripe_off[s + 1]],
            )

        def load_f(ci):
            # NOTE: f loads must stay on the sync queue with the x stripes —
            # sharing the scalar queue with the out stores corrupted results
            # on hardware (loads and stores on one HWDGE queue), though the
            # simulator was clean.
            w0, wc = CHUNKS[ci]
            lb = 15 * wc + 1
            t = fpool.tile([3, LBMAX], bf16)
            nc.sync.dma_start(t[0:3, 0:lb], fs3[0:3, 15 * w0 : 15 * w0 + lb])
            return t

        # Out-grid column offset: +2048 on upper-half cores (ids 4..7).
        pid = nc.partition_id(
            engines=[mybir.EngineType.DVE, mybir.EngineType.Pool]
        )
        o0 = (pid >= 4) * 2048

        f_tiles = {}

        # --- per-stage emitters -------------------------------------------
        def emit_bcast(ci):
            """PE: broadcast f chunk into PSUM; ACT: mirror the GPS tail
            into SBUF. Returns (f_ps, fc, split)."""
            w0, wc = CHUNKS[ci]
            lb = 15 * wc + 1
            s = int(lb * BOOST_DVE_FRAC)
            f_sb = f_tiles.pop(ci)
            f_ps = psum.tile([128, LBMAX], f32, tag="fps")
            off = 0
            while off < lb:
                n = min(512, lb - off)
                nc.tensor.matmul(
                    f_ps[:, off : off + n], ones, f_sb[0:3, off : off + n]
                )
                off += n
            fc = fcpool.tile([128, MIRMAX], f32, tag="fc")
            nc.scalar.copy(fc[:, : lb - s], f_ps[:, s:lb])
            return (f_ps, fc, s)

        def emit_boost(ci, fstate):
            """DVE head + GPS tail of b = x * f. Returns b tile."""
            f_ps, fc, s = fstate
            w0, wc = CHUNKS[ci]
            lb = 15 * wc + 1
            fb = 15 * w0
            b = bpool.tile([128, LBMAX], f32, tag="b")
            nc.gpsimd.tensor_tensor(
                b[:, s:lb], xs_sb[:, fb + s : fb + lb], fc[:, : lb - s],
                mybir.AluOpType.mult,
            )
            nc.vector.tensor_tensor(
                b[:, :s], xs_sb[:, fb : fb + s], f_ps[:, :s],
                mybir.AluOpType.mult,
            )
            return b

        def emit_reduce(ci, b):
            """DVE: per-window max of chunk ci, written to PSUM so the
            is_equal broadcast read doesn't touch SBUF."""
            w0, wc = CHUNKS[ci]
            M = psum.tile([128, WCMAX], f32, tag="mps")
            nc.vector.tensor_reduce(
                M[:, :wc], win_view(b, 0, wc), axis=mybir.AxisListType.X,
                op=mybir.AluOpType.max,
            )
            return M

        def emit_mask(ci, b, M):
            """DVE: winner mask on the 16-grid via is_equal."""
            w0, wc = CHUNKS[ci]
            o = opool.tile([128, 16 * WCMAX], f32, tag="o")
            M3 = M[:, :wc].unsqueeze(2).broadcast_to([128, wc, DPC])
            nc.vector.tensor_tensor(
                grid_view(o, 0, wc), win_view(b, 0, wc), M3,
                mybir.AluOpType.is_equal,
            )
            return o

        def emit_outmul(ci, o):
            """GPS: o *= x on the out grid, then ACT DMAs it out."""
            w0, wc = CHUNKS[ci]
            ow = 16 * wc
            xo0 = o0 + 16 * w0
            nc.gpsimd.tensor_tensor(
                o[:, :ow], o[:, :ow], xs_sb[:, bass.ds(xo0, ow)],
                mybir.AluOpType.mult,
            )
            nc.scalar.dma_start(out[:, 16 * w0 : 16 * w0 + ow], o[:, :ow])

        # --- software-pipelined main loop ---------------------------------
        # At iteration i (chunks advance left to right):
        #   PE/ACT prepare chunk i+1;  boost chunk i;  reduce chunk i-1;
        #   mask chunk i-2;  out-multiply + store chunk i-3.
        def xs_need(i):
            """Leading xs column required before iteration i runs: the
            boost read of chunk i and the out-grid read of chunk i-3
            (+2048 worst-case upper-half offset)."""
            need = 0
            if i < NCH:
                w0, wc = CHUNKS[i]
                need = max(need, 15 * (w0 + wc) + 1)
            if 3 <= i and i - 3 < NCH:
                w0, wc = CHUNKS[i - 3]
                need = max(need, 2048 + 16 * (w0 + wc))
            return need

        f_tiles[0] = load_f(0)
        f_tiles[1] = load_f(1)
        f_tiles[2] = load_f(2)
        load_stripe(0)
        load_stripe(1)
        next_stripe = 2
        fstate = {0: emit_bcast(0)}
        bmap, Mmap, omap = {}, {}, {}

        for i in range(NCH + 3):
            if i + 3 < NCH:
                f_tiles[i + 3] = load_f(i + 3)
            # keep xs stripes three iterations ahead of consumption: cores
            # with slower HBM arbitration (6/7 in traces) stall GPS waiting
            # for x columns under just-in-time pacing, while fully
            # front-loading the reads starves the store queue instead.
            need = max(xs_need(i + 1), xs_need(i + 2), xs_need(i + 3))
            target = 0
            while target < len(STRIPES) - 1 and stripe_off[target + 1] < need:
                target += 1
            if i + 1 >= NCH + 2:
                target = len(STRIPES) - 1
            while next_stripe <= target:
                load_stripe(next_stripe)
                next_stripe += 1
            if i + 1 < NCH:
                fstate[i + 1] = emit_bcast(i + 1)
            # GPS runs the out-multiply FIRST each iteration so it overlaps
            # DVE's PSUM-fed boost head instead of the SBUF-heavy is_equal
            # (strided window reads degrade badly under SBUF contention).
            if 3 <= i:
                emit_outmul(i - 3, omap.pop(i - 3))
            if i < NCH:
                bmap[i] = emit_boost(i, fstate.pop(i))
            if 1 <= i < NCH + 1:
                Mmap[i - 1] = emit_reduce(i - 1, bmap[i - 1])
            if 2 <= i < NCH + 2:
                omap[i - 2] = emit_mask(i - 2, bmap[i - 2], Mmap.pop(i - 2))
                del bmap[i - 2]

    nc.compile()
    return nc


def _get_nc():
    if "nc" not in _CACHE:
        _CACHE["nc"] = _build()
    return _CACHE["nc"]


def _split_bf16_3(f):
    """Split fp32 f into three bf16 terms whose fp32 sum is exactly f
    (verified by assertion, in both association orders)."""
    import ml_dtypes

    bf = ml_dtypes.bfloat16
    hi = f.astype(bf)
    r = (f - hi.astype(np.float32)).astype(np.float32)
    mid = r.astype(bf)
    r2 = (r - mid.astype(np.float32)).astype(np.float32)
    lo = r2.astype(bf)
    f32 = np.float32
    assert np.array_equal((hi.astype(f32) + mid.astype(f32)) + lo.astype(f32), f)
    assert np.array_equal(hi.astype(f32) + (mid.astype(f32) + lo.astype(f32)), f)
    return np.stack([hi, mid, lo], axis=0)


def _shard_inputs(x, duty_cycle, boost_strength):
    x = np.ascontiguousarray(x, dtype=np.float32)
    duty = np.asarray(duty_cycle, dtype=np.float32)
    bs = np.asarray(boost_strength, dtype=np.float32)

    # Boost factors, matching the reference's fp32 arithmetic: the product
    # is computed in fp32 exactly as jax does; exp is evaluated in float64
    # and rounded once to fp32 (correctly-rounded expf).
    t = (np.float32(OUT_DIM / N) - duty) * bs[0]
    f = np.exp(t.astype(np.float64)).astype(np.float32)

    in_maps = []
    for i in range(NCORES):
        q, h = i % 4, i // 4
        xs_i = np.ascontiguousarray(
            x[128 * q : 128 * (q + 1), 30720 * h : 30720 * h + XS_COLS]
        )
        fs_i = _split_bf16_3(
            np.ascontiguousarray(f[30720 * h : 30720 * h + FS_COLS])
        )
        in_maps.append({"xs": xs_i, "fs3": fs_i})
    return in_maps


def _assemble(results):
    full = np.empty((BATCH, N), dtype=np.float32)
    for i in range(NCORES):
        q, h = i % 4, i // 4
        full[
            128 * q : 128 * (q + 1), 32768 * h : 32768 * h + OUT_COLS
        ] = results[i]["out"]
    return full


def kernel(x, duty_cycle, boost_strength):
    from concourse.bass_utils import run_bass_kernel_spmd

    nc = _get_nc()
    in_maps = _shard_inputs(x, duty_cycle, boost_strength)
    res = run_bass_kernel_spmd(nc, in_maps, list(range(NCORES)))
    return _assemble(res.results)


# revision 28
# speedup vs baseline: 1.0270x; 1.0270x over previous
"""Trainium2 Bass kernel for DKWinners (overlapping-window k-winners masking).

Problem: x [512, 65536] f32. boosted = x * exp((1/16 - duty_cycle) * bs).
For each of 4096 windows (window k covers boosted cols [15k, 15k+16)),
find the max; the output keeps x[:, 16k+j] where j is the argmax slot of
window k (mask laid on the non-overlapping 16-grid) and zeros the rest.

Sharding (8 cores): 4 batch-quarters x 2 column-halves. Each core handles
128 batch rows (= the 128 SBUF partitions) and 2048 windows (one column
half). Column half h covers boosted cols [30720h, 30720h+30721) and out
cols [32768h, 32768h+32768); the per-core x shard is x[:, 30720h : +34816]
which contains both.

Engine schedule: a 4-stage software pipeline over window chunks with a
3-iteration stagger so no engine ever waits on another engine's output
from the same iteration:
  stage 1 (iter i):   PE broadcasts boost factors for chunk i+1 into PSUM
                      (K=3 bf16 matmul, exact fp32 sum); ACT mirrors the
                      GPS-assigned tail into SBUF (GPS can't read PSUM).
  stage 2 (iter i):   DVE computes the boost head b=x*f from PSUM, GPS the
                      tail from the ACT mirror (split tuned so both
                      engines finish together).
  stage 3 (iter i):   DVE reduces chunk i-1's windows to maxes M (strided
                      3D view) and writes chunk i-2's winner mask o via
                      is_equal(b_win, M broadcast) on the 16-grid
                      (fp32-exact, so equality == argmax).
  stage 4 (iter i):   GPS multiplies chunk i-3's mask by x in place; ACT's
                      HWDGE queue DMAs it out.
All x stripes are issued up front on the sync queue (SBUF holds the whole
shard); out stores go on ACT's queue so loads and stores stream on
different queues. The out grid sits +2048 cols relative to the boost grid
on upper-half cores; that offset is runtime-computed from partition_id so
one SPMD program serves all 8 cores.
"""
import numpy as np
from contextlib import ExitStack

BATCH = 512
N = 65536
OUT_DIM = 4096
DPC = 16
NCORES = 8

H_WINDOWS = 2048          # windows per column half
# 96-window chunks keep the f broadcast within 3 PSUM banks so the window
# maxes M fit in the remaining 2 banks (is_equal's stride-0 broadcast of M
# then reads PSUM, not SBUF — SBUF-port contention with GPS made it 1.7x
# slower otherwise).
WCMAX = 96
# chunk plan: (window_start, window_count); tapered at the start to fill
# the pipeline quickly
_sizes = [32, 64] + [96] * 20 + [32]
assert sum(_sizes) == H_WINDOWS
CHUNKS = []
_w = 0
for _s in _sizes:
    CHUNKS.append((_w, _s))
    _w += _s
NCH = len(CHUNKS)
LBMAX = 15 * WCMAX + 1
BOOST_DVE_FRAC = 0.55     # fraction of boost cols multiplied on DVE
OUT_GPS_FRAC = 1.0        # fraction of out windows multiplied on GPS
XS_COLS = 34816           # per-core x shard cols
OUT_COLS = 32768          # per-core out cols
FS_COLS = 15 * H_WINDOWS + 1  # boost-factor cols per half (30721)
# x stripe plan: small leading stripes so chunk 0 starts early
STRIPES = [544, 1632] + [2176] * 15
assert sum(STRIPES) == XS_COLS

_CACHE: dict = {}


def _build():
    import concourse.bacc as bacc
    import concourse.bass as bass
    import concourse.mybir as mybir
    import concourse.tile as tile
    from concourse.ap import AP

    f32 = mybir.dt.float32
    bf16 = mybir.dt.bfloat16

    nc = bacc.Bacc(
        "TRN2", target_bir_lowering=False, debug=False, num_devices=NCORES
    )
    xs = nc.dram_tensor("xs", [128, XS_COLS], f32, kind="ExternalInput")
    fs3 = nc.dram_tensor("fs3", [3, FS_COLS], bf16, kind="ExternalInput")
    out = nc.dram_tensor("out", [128, OUT_COLS], f32, kind="ExternalOutput")

    def win_view(ap, w0, nwin):
        """[128, nwin, 16] overlapping-window view (stride 15), starting at
        window w0 of the buffer AP."""
        base = ap[:, 15 * w0 : 15 * w0 + 15 * nwin + 1]
        return AP(base.tensor, base.offset,
                  [list(base.ap[0]), [15, nwin], [1, DPC]])

    def grid_view(ap, w0, nwin):
        """[128, nwin, 16] contiguous 16-grid view starting at window w0."""
        base = ap[:, 16 * w0 : 16 * (w0 + nwin)]
        return AP(base.tensor, base.offset,
                  [list(base.ap[0]), [DPC, nwin], [1, DPC]])

    MIRMAX = LBMAX - int(LBMAX * BOOST_DVE_FRAC) + 8

    with tile.TileContext(nc) as tc, ExitStack() as ctx:
        bpool = ctx.enter_context(tc.tile_pool(name="b", bufs=3))
        opool = ctx.enter_context(tc.tile_pool(name="o", bufs=4))
        fpool = ctx.enter_context(tc.tile_pool(name="f", bufs=3))
        fcpool = ctx.enter_context(tc.tile_pool(name="fc", bufs=2))
        mpool = ctx.enter_context(tc.tile_pool(name="m", bufs=3))
        dpool = ctx.enter_context(tc.tile_pool(name="d", bufs=3))
        psum = ctx.enter_context(tc.tile_pool(name="ps", bufs=2, space="PSUM"))

        # Static allocations outside the pools (no pool padding).
        xs_sb = nc.alloc_sbuf_tensor("xs_sb", [128, XS_COLS], f32).ap()
        ones = nc.alloc_sbuf_tensor("ones_sb", [3, 128], bf16).ap()
        nc.vector.memset(ones, 1.0)
        zeros = nc.alloc_sbuf_tensor("zeros_sb", [128, 1], f32).ap()
        nc.vector.memset(zeros, 0.0)

        stripe_off = [0]
        for w in STRIPES:
            stripe_off.append(stripe_off[-1] + w)

        def load_stripe(s):
            nc.sync.dma_start(
                xs_sb[:, stripe_off[s] : stripe_off[s + 1]],
                xs[:, stripe_off[s] : stripe_off[s + 1]],
            )

        def load_f(ci):
            # NOTE: f loads must stay on the sync queue with the x stripes —
            # sharing the scalar queue with the out stores corrupted results
            # on hardware (loads and stores on one HWDGE queue), though the
            # simulator was clean.
            w0, wc = CHUNKS[ci]
            lb = 15 * wc + 1
            t = fpool.tile([3, LBMAX], bf16)
            nc.sync.dma_start(t[0:3, 0:lb], fs3[0:3, 15 * w0 : 15 * w0 + lb])
            return t

        # Out-grid column offset: +2048 on upper-half cores (ids 4..7).
        pid = nc.partition_id(
            engines=[mybir.EngineType.DVE, mybir.EngineType.Pool,
                     mybir.EngineType.Activation]
        )
        o0 = (pid >= 4) * 2048

        f_tiles = {}

        # --- per-stage emitters -------------------------------------------
        def emit_bcast(ci):
            """PE: broadcast f chunk into PSUM; ACT: mirror the GPS tail
            into SBUF. Returns (f_ps, fc, split)."""
            w0, wc = CHUNKS[ci]
            lb = 15 * wc + 1
            s = int(lb * BOOST_DVE_FRAC)
            f_sb = f_tiles.pop(ci)
            f_ps = psum.tile([128, LBMAX], f32, tag="fps")
            off = 0
            while off < lb:
                n = min(512, lb - off)
                nc.tensor.matmul(
                    f_ps[:, off : off + n], ones, f_sb[0:3, off : off + n]
                )
                off += n
            fc = fcpool.tile([128, MIRMAX], f32, tag="fc")
            nc.scalar.copy(fc[:, : lb - s], f_ps[:, s:lb])
            return (f_ps, fc, s)

        def emit_boost(ci, fstate):
            """DVE head + GPS tail of b = x * f. Returns b tile."""
            f_ps, fc, s = fstate
            w0, wc = CHUNKS[ci]
            lb = 15 * wc + 1
            fb = 15 * w0
            b = bpool.tile([128, LBMAX], f32, tag="b")
            nc.gpsimd.tensor_tensor(
                b[:, s:lb], xs_sb[:, fb + s : fb + lb], fc[:, : lb - s],
                mybir.AluOpType.mult,
            )
            nc.vector.tensor_tensor(
                b[:, :s], xs_sb[:, fb : fb + s], f_ps[:, :s],
                mybir.AluOpType.mult,
            )
            return b

        def emit_reduce(ci, b):
            """DVE: per-window max of chunk ci (SBUF: GPS reads the M
            broadcast for the subtract and cannot read PSUM)."""
            w0, wc = CHUNKS[ci]
            M = mpool.tile([128, WCMAX], f32, tag="m")
            nc.vector.tensor_reduce(
                M[:, :wc], win_view(b, 0, wc), axis=mybir.AxisListType.X,
                op=mybir.AluOpType.max,
            )
            return M

        def emit_xcopy(ci):
            """ACT: prefill the output tile with x on the out grid."""
            w0, wc = CHUNKS[ci]
            ow = 16 * wc
            xo0 = o0 + 16 * w0
            o = opool.tile([128, 16 * WCMAX], f32, tag="o")
            nc.scalar.copy(o[:, :ow], xs_sb[:, bass.ds(xo0, ow)])
            return o

        def emit_sub(ci, b, M):
            """GPS: d = M3 - b_win on the 16-grid; d == 0 exactly at the
            winner slot (bf16 write is safe: a nonzero normal fp32
            difference never rounds to bf16 zero)."""
            w0, wc = CHUNKS[ci]
            d = dpool.tile([128, 16 * WCMAX], bf16, tag="d")
            M3 = M[:, :wc].unsqueeze(2).broadcast_to([128, wc, DPC])
            nc.gpsimd.tensor_tensor(
                grid_view(d, 0, wc), M3, win_view(b, 0, wc),
                mybir.AluOpType.subtract,
            )
            return d

        def emit_cpred(ci, o, d):
            """DVE: zero the non-winners (d != 0) in the x-prefilled
            output tile, then ACT DMAs it out."""
            w0, wc = CHUNKS[ci]
            ow = 16 * wc
            Z = zeros.broadcast_to([128, ow])
            # cpred requires an integer mask; bitcast the bf16 difference
            # (+0.0 is bit pattern 0, and d >= 0 so no -0.0 cases).
            nc.vector.copy_predicated(
                o[:, :ow], d[:, :ow].bitcast(mybir.dt.uint16), Z)
            nc.scalar.dma_start(out[:, 16 * w0 : 16 * w0 + ow], o[:, :ow])

        # --- software-pipelined main loop ---------------------------------
        # At iteration i (chunks advance left to right):
        #   PE/ACT prepare chunk i+1;  boost chunk i;  reduce chunk i-1;
        #   mask chunk i-2;  out-multiply + store chunk i-3.
        def xs_need(i):
            """Leading xs column required before iteration i runs: the
            boost read of chunk i and the out-grid read of chunk i-3
            (+2048 worst-case upper-half offset)."""
            need = 0
            if i < NCH:
                w0, wc = CHUNKS[i]
                need = max(need, 15 * (w0 + wc) + 1)
            if 3 <= i and i - 3 < NCH:
                w0, wc = CHUNKS[i - 3]
                need = max(need, 2048 + 16 * (w0 + wc))
            return need

        f_tiles[0] = load_f(0)
        f_tiles[1] = load_f(1)
        f_tiles[2] = load_f(2)
        load_stripe(0)
        load_stripe(1)
        next_stripe = 2
        fstate = {0: emit_bcast(0)}
        bmap, Mmap, omap, dmap = {}, {}, {}, {}

        for i in range(NCH + 3):
            if i + 3 < NCH:
                f_tiles[i + 3] = load_f(i + 3)
            # keep xs stripes three iterations ahead of consumption: cores
            # with slower HBM arbitration (6/7 in traces) stall GPS waiting
            # for x columns under just-in-time pacing, while fully
            # front-loading the reads starves the store queue instead.
            need = max(xs_need(i + 1), xs_need(i + 2), xs_need(i + 3))
            target = 0
            while target < len(STRIPES) - 1 and stripe_off[target + 1] < need:
                target += 1
            if i + 1 >= NCH + 2:
                target = len(STRIPES) - 1
            while next_stripe <= target:
                load_stripe(next_stripe)
                next_stripe += 1
            if i + 1 < NCH:
                fstate[i + 1] = emit_bcast(i + 1)
            if i < NCH:
                bmap[i] = emit_boost(i, fstate.pop(i))
            if 1 <= i < NCH + 1:
                Mmap[i - 1] = emit_reduce(i - 1, bmap[i - 1])
                omap[i - 1] = emit_xcopy(i - 1)
            if 2 <= i < NCH + 2:
                dmap[i - 2] = emit_sub(i - 2, bmap[i - 2], Mmap.pop(i - 2))
                del bmap[i - 2]
            if 3 <= i:
                emit_cpred(i - 3, omap.pop(i - 3), dmap.pop(i - 3))

    nc.compile()
    return nc


def _get_nc():
    if "nc" not in _CACHE:
        _CACHE["nc"] = _build()
    return _CACHE["nc"]


def _split_bf16_3(f):
    """Split fp32 f into three bf16 terms whose fp32 sum is exactly f
    (verified by assertion, in both association orders)."""
    import ml_dtypes

    bf = ml_dtypes.bfloat16
    hi = f.astype(bf)
    r = (f - hi.astype(np.float32)).astype(np.float32)
    mid = r.astype(bf)
    r2 = (r - mid.astype(np.float32)).astype(np.float32)
    lo = r2.astype(bf)
    f32 = np.float32
    assert np.array_equal((hi.astype(f32) + mid.astype(f32)) + lo.astype(f32), f)
    assert np.array_equal(hi.astype(f32) + (mid.astype(f32) + lo.astype(f32)), f)
    return np.stack([hi, mid, lo], axis=0)


def _shard_inputs(x, duty_cycle, boost_strength):
    x = np.ascontiguousarray(x, dtype=np.float32)
    duty = np.asarray(duty_cycle, dtype=np.float32)
    bs = np.asarray(boost_strength, dtype=np.float32)

    # Boost factors, matching the reference's fp32 arithmetic: the product
    # is computed in fp32 exactly as jax does; exp is evaluated in float64
    # and rounded once to fp32 (correctly-rounded expf).
    t = (np.float32(OUT_DIM / N) - duty) * bs[0]
    f = np.exp(t.astype(np.float64)).astype(np.float32)

    in_maps = []
    for i in range(NCORES):
        q, h = i % 4, i // 4
        xs_i = np.ascontiguousarray(
            x[128 * q : 128 * (q + 1), 30720 * h : 30720 * h + XS_COLS]
        )
        fs_i = _split_bf16_3(
            np.ascontiguousarray(f[30720 * h : 30720 * h + FS_COLS])
        )
        in_maps.append({"xs": xs_i, "fs3": fs_i})
    return in_maps


def _assemble(results):
    full = np.empty((BATCH, N), dtype=np.float32)
    for i in range(NCORES):
        q, h = i % 4, i // 4
        full[
            128 * q : 128 * (q + 1), 32768 * h : 32768 * h + OUT_COLS
        ] = results[i]["out"]
    return full


def kernel(x, duty_cycle, boost_strength):
    from concourse.bass_utils import run_bass_kernel_spmd

    nc = _get_nc()
    in_maps = _shard_inputs(x, duty_cycle, boost_strength)
    res = run_bass_kernel_spmd(nc, in_maps, list(range(NCORES)))
    return _assemble(res.results)
